# revision 4
# baseline (speedup 1.0000x reference)
"""DCNv2 (deformable conv v2) forward on 8 Trainium2 NeuronCores.

Problem (hardcoded): x [4,64,64,256] f32, offset_w [3,3,256,27], offset_b
[1,1,1,27], dcn_weight [256,256,3,3]; K=3, stride=1, padding=1.

Sharding: 8 cores = 4 images x 2 H-halves. Each core computes its half's
[32,64,256] output from a 45-row padded window of its image.

Per-core pipeline (single SPMD Bass program), bf16 hot path:
  A. om conv (PE) + position chain (DVE) + SWDGE-wrapped int16 gather
     index build (PE selection/replication matmuls) for all 2048 px.
  B. G-GEMM (PE, bf16): G[t,s,f] = pad_x @ W2 streamed through a 6-deep
     [128,512] PSUM ring; stored to DRAM in a paired-row layout
     G2[r] = [G[r] | G[r+67]] so one 1024-element gather unit holds all
     4 bilinear corners of a tap.
  C. per 128-px tile: one 1152-index dma_gather (GPSIMD ucode, mlp
     library) double-buffered against the 36-corner weighted accumulate
     (DVE scalar_tensor_tensor chains + ACT products for 2 taps).
"""

import sys

sys.path.insert(0, "/opt/trn_rl_repo")

import numpy as np
import ml_dtypes

_KERNEL_CACHE = {}

# ---------------- problem geometry (hardcoded) ----------------
N, H, W, C = 4, 64, 64, 256
KK = 9
OUTC = 256
PW = 67            # padded width/height (pad (1,2))
ROWS = 45          # rows per core window: [h0-7, h0+37]
SW = 3072          # padded window row-pitch (45*67=3015, padded to 24*128)
NCHUNK = SW // 128  # 24 G s-chunks
GROWS = KK * SW     # 27648 G rows
PXT = 16           # 128-px tiles per core
NPX = 2048         # pixels per core
CLIP_HI = 65.0     # reference clips pos to [0, H+2p-1] = [0, 65]
# GPSIMD runs only the gathers (mlp library); DVE accumulates in three
# 12-corner bf16 chains (shorter chains = less bf16 rounding drift)
DVE_TAPS_A = (0, 1)
DVE_TAPS_B = (2, 3)
ACT_TAPS = (4, 5, 6, 7, 8)


def _build_program():
    import concourse.bacc as bacc
    import concourse.mybir as mybir
    import concourse.tile as tile

    f32 = mybir.dt.float32
    bf16 = mybir.dt.bfloat16
    i16 = mybir.dt.int16
    i32 = mybir.dt.int32

    nc = bacc.Bacc()

    px_in = nc.dram_tensor("px", [2, 128, SW], bf16, kind="ExternalInput")
    w2_in = nc.dram_tensor("w2", [2, 128, KK * 256], bf16, kind="ExternalInput")
    ow_in = nc.dram_tensor("ow", [128, 18 * 27], bf16, kind="ExternalInput")
    bi_in = nc.dram_tensor("bi", [PXT, 128, 18], f32, kind="ExternalInput")
    mb_in = nc.dram_tensor("mb", [128, 9], f32, kind="ExternalInput")
    ic_in = nc.dram_tensor("ic", [128, 18], f32, kind="ExternalInput")
    id_in = nc.dram_tensor("id01", [128, 128], f32, kind="ExternalInput")
    rp_in = nc.dram_tensor("rp", [16, 128], f32, kind="ExternalInput")
    out_d = nc.dram_tensor("out", [NPX, 256], f32, kind="ExternalOutput")
    import os as _os
    _dbg = _os.environ.get("DCN_DEBUG") == "1"
    if _dbg:
        dbg_idxw = nc.dram_tensor("dbg_idxw", [128, 4 * 288], i16,
                                  kind="ExternalOutput")
        dbg_w4 = nc.dram_tensor("dbg_w4", [128, 4 * 144], f32,
                                kind="ExternalOutput")
        dbg_gout = nc.dram_tensor("dbg_gout", [128, 18 * 512], f32,
                                  kind="ExternalOutput")
        dbg_gd = nc.dram_tensor("dbg_gd", [256, 256], f32,
                                kind="ExternalOutput")
        dbg_om = nc.dram_tensor("dbg_om", [128, 4 * 108], f32,
                                kind="ExternalOutput")
        dbg_pos = nc.dram_tensor("dbg_pos", [128, 4 * 72], f32,
                                 kind="ExternalOutput")

    with tile.TileContext(nc) as tc:
        with (
            tc.tile_pool(name="cst", bufs=1) as cst,
            tc.tile_pool(name="dr", bufs=1, space="DRAM") as dr,
        ):
            # ---- persistent inputs ----
            px_b = cst.tile([128, 2 * SW], bf16)
            for cc in range(2):
                nc.sync.dma_start(
                    out=px_b[:, cc * SW:(cc + 1) * SW], in_=px_in[cc, :, :])
            w2_b = cst.tile([128, 2 * KK * 256], bf16)
            for cc in range(2):
                nc.sync.dma_start(
                    out=w2_b[:, cc * 2304:(cc + 1) * 2304], in_=w2_in[cc, :, :])
            ow_b = cst.tile([128, 18 * 27], bf16)
            nc.sync.dma_start(out=ow_b[:, :], in_=ow_in[:, :])
            mb_t = cst.tile([128, 9], f32)
            nc.sync.dma_start(out=mb_t[:, :], in_=mb_in[:, :])
            ic_t = cst.tile([128, 18], f32)
            nc.sync.dma_start(out=ic_t[:, :], in_=ic_in[:, :])
            id_t = cst.tile([128, 128], f32)
            nc.sync.dma_start(out=id_t[:, :], in_=id_in[:, :])
            rp_t = cst.tile([16, 128], f32)
            nc.sync.dma_start(out=rp_t[:16, :], in_=rp_in[:, :])
            # persisted per-pixel combine weights / gather indices
            w4_all = cst.tile([128, 4 * 144], f32)
            # int16 gather indices, SWDGE-wrapped layout: per c4 576 cols
            # (j, t-pair, cc); value for pixel cc*16+q sits at [q, j*144+t*8+cc]
            # replicated across the eight 16-partition groups.
            idxw = cst.tile([128, 4 * 288], i16)

            gdram = dr.tile([GROWS, 512], bf16)
            with tc.tile_pool(name="wrk", bufs=1) as wrk:

              # ---- phase A: om conv + position chain, all 4 c4 blocks ---
              with tc.tile_pool(name="ppA", bufs=1, space="PSUM") as pp:
                for c4 in range(4):
                    bi_t = wrk.tile([128, 4 * 18], f32, tag="bi")
                    nc.sync.dma_start(
                        out=bi_t[:, :].rearrange("p (j t) -> p j t", j=4),
                        in_=bi_in[4 * c4:4 * c4 + 4, :, :].transpose((1, 0, 2)),
                    )
                    om_ps = pp.tile([27, 512], f32, tag="om", bufs=2)
                    for s in range(9):
                        for cc in range(2):
                            k2 = s * 2 + cc
                            di, dj = s // 3, s % 3
                            base = cc * SW + (7 + 8 * c4 + di) * PW + dj
                            rhs = px_b[:, base:base + 8 * PW].rearrange(
                                "p (h w) -> p h w", h=8, w=PW)[:, :, 0:64]
                            nc.tensor.matmul(
                                out=om_ps[:, :],
                                lhsT=ow_b[:, 27 * k2:27 * k2 + 27],
                                rhs=rhs,
                                start=(k2 == 0),
                                stop=(k2 == 17),
                            )
                    om_sb = wrk.tile([27, 512], f32, tag="omsb", bufs=2)
                    nc.vector.tensor_copy(out=om_sb[:, :], in_=om_ps[:, :])
                    omt = wrk.tile([128, 4 * 27], f32, tag="omt")
                    for j in range(4):
                        tp = pp.tile([128, 27], f32, tag="tp")
                        nc.tensor.transpose(
                            out=tp[:, :], in_=om_sb[:, 128 * j:128 * j + 128],
                            identity=id_t[:27, :27],
                        )
                        nc.vector.tensor_copy(
                            out=omt[:, 27 * j:27 * j + 27], in_=tp[:, :])

                    # ---- position chain ([128, (4j, .)] strided APs) ----
                    def om_ap(lo, n):
                        return omt[:, :].rearrange("p (j t) -> p j t", j=4)[:, :, lo:lo + n]

                    if _dbg:
                        nc.sync.dma_start(
                            out=dbg_om[:, 108 * c4:108 * c4 + 108],
                            in_=omt[:, :])

                    pos = wrk.tile([128, 72], f32, tag="pos")
                    pos3 = pos[:, :].rearrange("p (j t) -> p j t", j=4)
                    nc.vector.tensor_tensor(
                        out=pos3, in0=om_ap(0, 18),
                        in1=bi_t[:, :].rearrange("p (j t) -> p j t", j=4),
                        op=mybir.AluOpType.add,
                    )
                    if _dbg:
                        nc.sync.dma_start(
                            out=dbg_pos[:, 72 * c4:72 * c4 + 72],
                            in_=pos[:, :])
                    nc.vector.tensor_scalar(
                        out=pos[:, :], in0=pos[:, :], scalar1=0.0, scalar2=CLIP_HI,
                        op0=mybir.AluOpType.max, op1=mybir.AluOpType.min,
                    )
                    ymx = wrk.tile([128, 72], f32, tag="ymx")
                    nc.vector.tensor_scalar(
                        out=ymx[:, :], in0=pos[:, :], scalar1=-0.5, scalar2=None,
                        op0=mybir.AluOpType.add,
                    )
                    yi = wrk.tile([128, 72], i32, tag="yi")
                    nc.vector.tensor_copy(out=yi[:, :], in_=ymx[:, :])
                    yf = wrk.tile([128, 72], f32, tag="yf")
                    nc.vector.tensor_copy(out=yf[:, :], in_=yi[:, :])
                    lo = wrk.tile([128, 72], f32, tag="lo")
                    nc.vector.tensor_tensor(
                        out=lo[:, :], in0=pos[:, :], in1=yf[:, :],
                        op=mybir.AluOpType.subtract,
                    )
                    hi = wrk.tile([128, 72], f32, tag="hi")
                    nc.vector.tensor_scalar(
                        out=hi[:, :], in0=lo[:, :], scalar1=-1.0, scalar2=1.0,
                        op0=mybir.AluOpType.mult, op1=mybir.AluOpType.add,
                    )
                    mk = wrk.tile([128, 36], f32, tag="mk")
                    mk3 = mk[:, :].rearrange("p (j t) -> p j t", j=4)
                    nc.vector.tensor_tensor(
                        out=mk3, in0=om_ap(18, 9),
                        in1=mb_t[:, :].unsqueeze(1).broadcast_to([128, 4, 9]),
                        op=mybir.AluOpType.add,
                    )
                    nc.scalar.activation(
                        out=mk[:, :], in_=mk[:, :],
                        func=mybir.ActivationFunctionType.Sigmoid,
                    )

                    def part(t, lo_, n):
                        return t[:, :].rearrange("p (j c) -> p j c", j=4)[:, :, lo_:lo_ + n]

                    mh = wrk.tile([128, 36], f32, tag="mh")
                    nc.vector.tensor_tensor(
                        out=mh[:, :].rearrange("p (j c) -> p j c", j=4),
                        in0=mk3, in1=part(hi, 0, 9),
                        op=mybir.AluOpType.mult,
                    )
                    ml = wrk.tile([128, 36], f32, tag="ml")
                    nc.vector.tensor_tensor(
                        out=ml[:, :].rearrange("p (j c) -> p j c", j=4),
                        in0=mk3, in1=part(lo, 0, 9),
                        op=mybir.AluOpType.mult,
                    )
                    w44 = w4_all[:, 144 * c4:144 * c4 + 144].rearrange(
                        "p (j t q) -> p j t q", q=4, j=4)
                    for q, (ay, ax) in enumerate(
                        [(mh, 0), (mh, 1), (ml, 0), (ml, 1)]
                    ):
                        src = part(hi if ax == 0 else lo, 9, 9)
                        eng = nc.vector
                        eng.tensor_tensor(
                            out=w44[:, :, :, q],
                            in0=ay[:, :].rearrange("p (j c) -> p j c", j=4),
                            in1=src,
                            op=mybir.AluOpType.mult,
                        )
                    rr = wrk.tile([128, 36], f32, tag="rr")
                    rr3 = rr[:, :].rearrange("p (j c) -> p j c", j=4)
                    nc.vector.tensor_scalar(
                        out=rr3, in0=part(yf, 0, 9),
                        scalar1=float(PW), scalar2=None, op0=mybir.AluOpType.mult,
                    )
                    nc.vector.tensor_tensor(
                        out=rr3, in0=rr3, in1=part(yf, 9, 9),
                        op=mybir.AluOpType.add,
                    )
                    idxf = wrk.tile([128, 36], f32, tag="idxf")
                    nc.vector.tensor_tensor(
                        out=idxf[:, :].rearrange("p (j t) -> p j t", j=4),
                        in0=rr[:, :].rearrange("p (j t) -> p j t", j=4),
                        in1=ic_t[:, 0:9].unsqueeze(1).broadcast_to([128, 4, 9]),
                        op=mybir.AluOpType.add,
                    )
                    nc.vector.tensor_scalar(
                        out=idxf[:, :], in0=idxf[:, :], scalar1=0.0,
                        scalar2=float(GROWS - 2),
                        op0=mybir.AluOpType.max, op1=mybir.AluOpType.min,
                    )
                    # ---- SWDGE-wrap the indices for dma_gather ----
                    # wrap16[q, j*72+t*8+cc] = idxf[cc*16+q, 9j+t]
                    wrap16 = wrk.tile([16, 288], f32, tag="wrap16")
                    for cc in range(8):
                        selp = pp.tile([16, 36], f32, tag="selp", bufs=2)
                        nc.tensor.matmul(
                            out=selp[:, :],
                            lhsT=id_t[:, 16 * cc:16 * cc + 16],
                            rhs=idxf[:, :],
                            start=True, stop=True,
                        )
                        nc.vector.tensor_copy(
                            out=wrap16[:16, :].rearrange(
                                "q (j t c) -> q j t c", j=4, c=8)[:, :, :, cc],
                            in_=selp[:16, :].rearrange("q (j t) -> q j t", j=4),
                        )
                    # replicate [16, 288] across the 8 partition groups
                    repp = pp.tile([128, 288], f32, tag="repp")
                    nc.tensor.matmul(
                        out=repp[:, :],
                        lhsT=rp_t[:16, :],
                        rhs=wrap16[:16, :],
                        start=True, stop=True,
                    )
                    nc.vector.tensor_copy(
                        out=idxw[:, 288 * c4:288 * c4 + 288], in_=repp[:, :])

              # ---- phase B: G-GEMM through a PSUM ring ----
              with tc.tile_pool(name="ppB", bufs=1, space="PSUM") as pp:
                for k in range(NCHUNK):
                    g_sb = wrk.tile([128, KK * 256], bf16, tag="gsb", bufs=2)
                    for g in range(5):  # tap groups of (2,2,2,2,1)
                        n = 512 if g < 4 else 256
                        gps = pp.tile([128, 512], f32, tag="gps", bufs=6)
                        for cc in range(2):
                            nc.tensor.matmul(
                                out=gps[:, 0:n],
                                lhsT=px_b[:, cc * SW + 128 * k:
                                          cc * SW + 128 * k + 128],
                                rhs=w2_b[:, cc * 2304 + g * 512:
                                         cc * 2304 + g * 512 + n],
                                start=(cc == 0),
                                stop=(cc == 1),
                            )
                        if (k * 5 + g) % 2 == 0:
                            nc.vector.tensor_copy(
                                out=g_sb[:, g * 512:g * 512 + n], in_=gps[:, 0:n])
                        else:
                            nc.scalar.activation(
                                out=g_sb[:, g * 512:g * 512 + n], in_=gps[:, 0:n],
                                func=mybir.ActivationFunctionType.Copy,
                            )
                    # paired-row G2 layout: G2[r] = [G[r] | G[r+67]], so one
                    # 1024-elem gather unit holds all 4 bilinear corners.
                    nc.sync.dma_start(
                        out=gdram[:, 0:256].rearrange("(t s) f -> s t f", t=KK)[
                            128 * k:128 * k + 128, :, :],
                        in_=g_sb[:, :].rearrange("p (t f) -> p t f", t=KK),
                    )
                    off = 67 - 128 * k if k == 0 else 0
                    # second pair-half write goes through the ACT HWDGE queue
                    # so the two G2 writes per chunk don't serialize on SP
                    nc.scalar.dma_start(
                        out=gdram[:, 256:512].rearrange(
                            "(t s) f -> s t f", t=KK)[
                            128 * k - 67 + off:128 * k + 61, :, :],
                        in_=g_sb[off:128, :].rearrange("p (t f) -> p t f", t=KK),
                    )

                # ---- phase C: batched gather + split combine, 16 tiles ----
                # overlapping [GROWS-1, 1024]-stride-512 view of gdram:
                # unit r = [G[r] | G[r+67] | G[r+1] | G[r+68]] = all 4 corners
                g_ov = gdram[:, :].copy()
                _v = g_ov.ap
                _v[0] = (512, GROWS - 1)
                _v[1] = (1, 1024)
                g_ov.ap = _v

                def emit_gather(j16):
                    gout = wrk.tile([128, 9, 1024], bf16, tag="gout", bufs=2)
                    nc.gpsimd.dma_gather(
                        out_ap=gout[:, :, :],
                        in_ap=g_ov,
                        idxs_ap=idxw[:, 72 * j16:72 * j16 + 72],
                        num_idxs=1152,
                        num_idxs_reg=1152,
                        elem_size=1024,
                        elem_step=512,
                        single_packet=False,
                    )
                    return gout

                def corner_w(j16, t_, yq, xq):
                    c4, j = j16 // 4, j16 % 4
                    col = 144 * c4 + 36 * j + 4 * t_ + 2 * yq + xq
                    return w4_all[:, col:col + 1]

                def chain(eng, gout, j16, taps, acc):
                    # fused multiply-accumulate into a pre-zeroed acc (an
                    # AP-scalar TENSOR_SCALAR costs ~1.8us on DVE; memset+stt
                    # is ~3x cheaper)
                    eng.memset(acc[:, :], 0.0)
                    for t_ in taps:
                        for yq in range(2):
                            for xq in range(2):
                                src = gout[:, t_, 512 * xq + 256 * yq:
                                           512 * xq + 256 * yq + 256]
                                wc = corner_w(j16, t_, yq, xq)
                                eng.scalar_tensor_tensor(
                                    out=acc[:, :], in0=src, scalar=wc,
                                    in1=acc[:, :],
                                    op0=mybir.AluOpType.mult,
                                    op1=mybir.AluOpType.add,
                                )

                def gp_chain(gout, j16, taps, tmp8, acc):
                    # Pool has no fused STT: per-corner tensor_scalar products
                    # into tmp8 slices, then a wide binary add tree.
                    g3 = gout
                    nq = 4 * len(taps)
                    for i, t_ in enumerate(taps):
                        for yq in range(2):
                            for xq in range(2):
                                src = g3[:, t_ * 2 + yq, 256 * xq:256 * xq + 256]
                                wc = corner_w(j16, t_, yq, xq)
                                sl = 4 * i + 2 * yq + xq
                                nc.gpsimd.tensor_scalar(
                                    out=tmp8[:, 256 * sl:256 * sl + 256],
                                    in0=src, scalar1=wc, scalar2=None,
                                    op0=mybir.AluOpType.mult,
                                )
                    width = nq * 256
                    while width > 256:
                        half = width // 2
                        nc.gpsimd.tensor_tensor(
                            out=tmp8[:, 0:half] if half > 256 else acc[:, :],
                            in0=tmp8[:, 0:half], in1=tmp8[:, half:width],
                            op=mybir.AluOpType.add,
                        )
                        width = half

                if _dbg:
                    nc.sync.dma_start(out=dbg_idxw[:, :], in_=idxw[:, :])
                    nc.sync.dma_start(out=dbg_w4[:, :], in_=w4_all[:, :])
                    for blk in range(2):
                        gsl = wrk.tile([128, 256], bf16, tag="gsl", bufs=2)
                        nc.sync.dma_start(
                            out=gsl[:, :],
                            in_=gdram[1340 + 128 * blk:1340 + 128 * blk + 128, :])
                        gslf = wrk.tile([128, 256], f32, tag="gslf", bufs=2)
                        nc.vector.tensor_copy(out=gslf[:, :], in_=gsl[:, :])
                        nc.sync.dma_start(
                            out=dbg_gd[128 * blk:128 * blk + 128, :],
                            in_=gslf[:, :])

                gouts = {0: emit_gather(0)}
                for j16 in range(PXT):
                    if j16 + 1 < PXT:
                        gouts[j16 + 1] = emit_gather(j16 + 1)
                    gout = gouts.pop(j16)
                    if _dbg and j16 == 0:
                        goutf = wrk.tile([128, 18 * 512], f32, tag="goutf")
                        nc.vector.tensor_copy(
                            out=goutf[:, :],
                            in_=gout[:, :, :].rearrange("p a b -> p (a b)"))
                        nc.sync.dma_start(out=dbg_gout[:, :], in_=goutf[:, :])
                    acc_a = wrk.tile([128, 256], bf16, tag="acca", bufs=2)
                    acc_b = wrk.tile([128, 256], bf16, tag="accb", bufs=2)
                    chain(nc.vector, gout, j16, DVE_TAPS_A, acc_a)
                    chain(nc.vector, gout, j16, DVE_TAPS_B, acc_b)
                    # taps 5-8: products on ACT (otherwise idle in phase C),
                    # summed by a wide binary tree on DVE
                    tc20 = wrk.tile([128, 20, 256], bf16, tag="tc20", bufs=2)
                    for i, (t_, yq, xq) in enumerate(
                        (t_, yq, xq) for t_ in ACT_TAPS
                        for yq in range(2) for xq in range(2)
                    ):
                        nc.scalar.activation(
                            out=tc20[:, i, :],
                            in_=gout[:, t_, 512 * xq + 256 * yq:
                                     512 * xq + 256 * yq + 256],
                            func=mybir.ActivationFunctionType.Copy,
                            scale=corner_w(j16, t_, yq, xq),
                        )
                    t4 = tc20[:, :, :].rearrange("p a b -> p (a b)")
                    # 20-slice binary reduction: 10, 5, then fold the tail
                    nc.vector.tensor_tensor(
                        out=t4[:, 0:2560], in0=t4[:, 0:2560],
                        in1=t4[:, 2560:5120], op=mybir.AluOpType.add,
                    )
                    nc.vector.tensor_tensor(
                        out=t4[:, 0:1280], in0=t4[:, 0:1280],
                        in1=t4[:, 1280:2560], op=mybir.AluOpType.add,
                    )
                    nc.vector.tensor_tensor(
                        out=t4[:, 0:512], in0=t4[:, 0:512],
                        in1=t4[:, 512:1024], op=mybir.AluOpType.add,
                    )
                    nc.vector.tensor_tensor(
                        out=t4[:, 0:256], in0=t4[:, 0:256],
                        in1=t4[:, 256:512], op=mybir.AluOpType.add,
                    )
                    nc.vector.tensor_tensor(
                        out=t4[:, 0:256], in0=t4[:, 0:256],
                        in1=t4[:, 1024:1280], op=mybir.AluOpType.add,
                    )
                    tmp = wrk.tile([128, 256], bf16, tag="tmp", bufs=2)
                    nc.vector.tensor_tensor(
                        out=tmp[:, :], in0=acc_a[:, :], in1=acc_b[:, :],
                        op=mybir.AluOpType.add,
                    )
                    osb = wrk.tile([128, 256], f32, tag="osb", bufs=2)
                    nc.vector.tensor_tensor(
                        out=osb[:, :], in0=tmp[:, :], in1=t4[:, 0:256],
                        op=mybir.AluOpType.add,
                    )
                    t0 = 128 * j16
                    nc.sync.dma_start(out=out_d[t0:t0 + 128, :], in_=osb[:, :])

    nc.compile()
    return nc


def _build_runner():
    import jax
    import concourse.mybir as mybir
    from jax.sharding import Mesh, NamedSharding, PartitionSpec
    from jax.experimental.shard_map import shard_map
    from concourse.bass2jax import (
        _bass_exec_p, partition_id_tensor, install_neuronx_cc_hook,
    )

    nc = _build_program()
    install_neuronx_cc_hook()
    n_cores = 8

    in_names, out_names, out_avals, zero_shapes = [], [], [], []
    partition_name = nc.partition_id_tensor.name if nc.partition_id_tensor else None
    for alloc in nc.m.functions[0].allocations:
        if not isinstance(alloc, mybir.MemoryLocationSet):
            continue
        name = alloc.memorylocations[0].name
        if alloc.kind == "ExternalInput":
            if name != partition_name:
                in_names.append(name)
        elif alloc.kind == "ExternalOutput":
            out_names.append(name)
            shape = tuple(alloc.tensor_shape)
            dtype = mybir.dt.np(alloc.dtype)
            out_avals.append(jax.core.ShapedArray(shape, dtype))
            zero_shapes.append((shape, dtype))
    n_params = len(in_names)
    n_outs = len(out_avals)
    all_in_names = in_names + out_names + ([partition_name] if partition_name else [])
    donate = tuple(range(n_params, n_params + n_outs))

    def _body(*args):
        operands = list(args)
        if partition_name is not None:
            operands.append(partition_id_tensor())
        return tuple(
            _bass_exec_p.bind(
                *operands,
                out_avals=tuple(out_avals),
                in_names=tuple(all_in_names),
                out_names=tuple(out_names),
                lowering_input_output_aliases=(),
                sim_require_finite=True,
                sim_require_nnan=True,
                nc=nc,
            )
        )

    devices = jax.devices()[:n_cores]
    mesh = Mesh(np.asarray(devices), ("core",))
    in_specs = (PartitionSpec("core"),) * (n_params + n_outs)
    out_specs = (PartitionSpec("core"),) * n_outs
    fn = jax.jit(
        shard_map(_body, mesh=mesh, in_specs=in_specs, out_specs=out_specs,
                  check_rep=False),
        donate_argnums=donate, keep_unused=True,
    )
    sh = NamedSharding(mesh, PartitionSpec("core"))

    def run(in_maps):
        ops = [
            jax.device_put(
                np.concatenate([np.asarray(m[n]) for m in in_maps], axis=0), sh)
            for n in in_names
        ]
        zeros = [
            jax.device_put(np.zeros((n_cores * s[0], *s[1:]), d), sh)
            for s, d in zero_shapes
        ]
        outs = [np.asarray(o) for o in fn(*ops, *zeros)]
        return [
            {n: outs[i].reshape(n_cores, *out_avals[i].shape)[c]
             for i, n in enumerate(out_names)}
            for c in range(n_cores)
        ]

    return run


def make_core_inputs(x, offset_w, offset_b, dcn_weight):
    """Build the 8 per-core input dicts (all host-side numpy)."""
    bf16 = ml_dtypes.bfloat16
    pad_x = np.pad(x, ((0, 0), (1, 2), (1, 2), (0, 0))).astype(np.float32)
    # W2 rows in (c, t) order to match the reference layout value[..., c*kk+t]
    w2 = dcn_weight.transpose(1, 2, 3, 0).reshape(C, KK, OUTC)  # [c, t, f]
    w2_arr = np.ascontiguousarray(
        w2.reshape(2, 128, KK * 256)).astype(bf16)
    # offset conv weights, channel order [y-offs(9), x-offs(9), mask(9)]
    perm = np.concatenate([np.arange(9) * 2, np.arange(9) * 2 + 1,
                           18 + np.arange(9)])
    owp = offset_w[..., perm]  # [3,3,256,27]
    ow_arr = np.zeros((128, 18 * 27), np.float32)
    for s in range(9):
        i, j = s // 3, s % 3
        for cc in range(2):
            k2 = s * 2 + cc
            ow_arr[:, 27 * k2:27 * k2 + 27] = owp[i, j, 128 * cc:128 * (cc + 1), :]
    ow_arr = ow_arr.astype(bf16)
    ob = offset_b.reshape(27)[perm]
    kr = np.array([-1.0, 0.0, 1.0], np.float32)
    inner_y = np.repeat(kr, 3)
    inner_x = np.tile(kr, 3)

    mb_arr = np.broadcast_to(ob[18:27][None, :], (128, 9)).astype(np.float32)
    id01 = np.eye(128, dtype=np.float32)
    rp_arr = np.zeros((16, 128), np.float32)
    for q in range(16):
        rp_arr[q, q::16] = 1.0

    in_maps = []
    for k in range(8):
        n, half = k // 2, k % 2
        h0 = 32 * half
        r0 = h0 - 7  # first padded row of the window
        # window rows [r0, r0+45), zero-padded outside [0, 67)
        win = np.zeros((ROWS, PW, C), np.float32)
        lo_ = max(0, r0)
        hi_ = min(PW, r0 + ROWS)
        win[lo_ - r0:hi_ - r0] = pad_x[n, lo_:hi_]
        pxt = win.transpose(2, 0, 1).reshape(C, ROWS * PW)
        px_arr = np.zeros((2, 128, SW), np.float32)
        px_arr[:, :, :ROWS * PW] = pxt.reshape(2, 128, ROWS * PW)
        px_arr = px_arr.astype(bf16)

        # baseinner [16, 128, 18]: global pos base per pixel/channel
        hloc = np.arange(32)
        wloc = np.arange(64)
        by = (h0 + hloc + 1).astype(np.float32)   # [32]
        bx = (wloc + 1).astype(np.float32)        # [64]
        bi_arr = np.zeros((PXT, 128, 18), np.float32)
        for t_ in range(PXT):
            byv = np.repeat(by[2 * t_:2 * t_ + 2], 64)  # [128]
            bxv = np.tile(bx, 2)                        # [128]
            bi_arr[t_, :, 0:9] = byv[:, None] + inner_y[None, :] + ob[0:9][None, :]
            bi_arr[t_, :, 9:18] = bxv[:, None] + inner_x[None, :] + ob[9:18][None, :]

        # gather consts: unit idx = t*SW + 67*(y1 - r0) + x1 (one per tap;
        # the paired-row G2 unit carries all 4 corners)
        ic_arr = np.zeros((128, 18), np.float32)
        for t_ in range(9):
            ic_arr[:, t_] = t_ * SW - PW * r0

        in_maps.append({
            "px": px_arr, "w2": w2_arr, "ow": ow_arr, "bi": bi_arr,
            "mb": mb_arr, "ic": ic_arr, "id01": id01, "rp": rp_arr,
        })
    return in_maps


def kernel(x, offset_w, offset_b, dcn_weight):
    x = np.asarray(x, np.float32)
    offset_w = np.asarray(offset_w, np.float32)
    offset_b = np.asarray(offset_b, np.float32)
    dcn_weight = np.asarray(dcn_weight, np.float32)

    if "run" not in _KERNEL_CACHE:
        _KERNEL_CACHE["run"] = _build_runner()
    run = _KERNEL_CACHE["run"]

    in_maps = make_core_inputs(x, offset_w, offset_b, dcn_weight)
    results = run(in_maps)

    y = np.zeros((N, H, W, OUTC), np.float32)
    for k in range(8):
        n, half = k // 2, k % 2
        y[n, 32 * half:32 * half + 32] = results[k]["out"].reshape(32, 64, OUTC)
    return y
